# revision 13
# baseline (speedup 1.0000x reference)
"""PointGroup clusters_voxelization kernel for Trainium2 (8 NeuronCores).

Strategy (sharding_hint): shard the 1024 clusters across 8 cores, 128
clusters each; feats/coords replicated. On each core, cluster c maps to
SBUF partition c, so all segment reductions (sum/min/max over the 2048
points of a cluster) are single-partition free-axis reductions.

v4 data movement (per core): the naive path (one indirect DMA per point
slot, 2048 calls) is bound by SWDGE descriptor generation on the Pool
engine (~1.4us/call -> ~2.9ms). Instead we use the batched dma_gather
custom instruction, which is ~9ns/index on one Q7 pair and scales ~3x
across the 4 SWDGE queues (one Q7 core pair each):

  - host builds a (N, 128) fp16 table (feats||coords padded to a 256B
    row, dma_gather requires 256B-aligned elements; fp16 costs ~3e-4
    rel err, budget is 2e-2)
  - dma_gather indices are int16 (<32768), so the table is processed as
    32 banks of 32768 rows; host re-sorts each cluster's points by bank
    and pads each (cluster, bank) cell to the bank-wide max count with
    duplicates of an existing member (min/max unaffected; the duplicate
    coordinate sum is passed in as a per-cluster correction so the mean
    stays exact)
  - gather calls (one per bank x 32-slot chunk, NI=4096 idx) round-robin
    the 4 SWDGE queues; gathered rows land in "slot" order
  - full rows stream out to a slot-ordered fp16 tensor; coords are cast
    to f32 on-chip; stats/transform as before; transformed coords out as
    a slot-ordered f32 tensor
  - host applies the inverse slot permutation per cluster and assembles
    feats(fp16->f32) || coords(f32)

The per-bank pad counts (Jb) are data-dependent and baked into the
program; the compiled program is cached keyed on them (one compile per
distinct input distribution; the numpy fallback covers degenerate data).
"""
import numpy as np

import concourse.bass as bass
import concourse.bacc as bacc
import concourse.tile as tile
import concourse.mybir as mybir
from concourse import bass_utils

N = 1048576
C = 32
NCLUSTER = 1024
PTS = 2048
S = NCLUSTER * PTS
NCORES = 8
P = 128                      # partitions = clusters per core
PPC = S // NCORES            # points per core = 262144
ROW = C + 3                  # 35 real elements per row
EROW = 128                   # padded fp16 row (256B) for dma_gather
BANKS = 32
BSHIFT = 15                  # 32768 rows per bank
SPAN = 64                    # slots per gather call (NI = SPAN*128 = 8192)
NQ = 4                       # SWDGE queues (Q7 core pairs)

_CACHE = {}
_PLAN_CACHE = {}

# set by kernel_timing.profile() to capture an NTFF trace on the next run
TRACE = False
LAST_RESULTS = None


def _dma_gather_raw(nc, out_ap, in_ap, idxs_ap, num_idxs, elem_size, elem_step, queue_num):
    """bass.dma_gather without the elem_size%256 assert.

    The Q7 ucode only requires the row STRIDE to be a multiple of 256B
    (address math is idx * stride_bytes_256 * 256); the payload length is
    free for non-transpose gathers, so we fetch just the 35 real fp16
    elements (70B) of each 256B-aligned table row."""
    eng = nc.gpsimd
    stride_bytes = elem_step * mybir.dt.size(in_ap.dtype)
    stride_bytes_256 = stride_bytes // 256
    assert stride_bytes % 256 == 0 and stride_bytes_256 < 256
    assert in_ap.ap[0][0] == elem_step
    assert in_ap.ap[-1][1] == elem_size
    assert out_ap.ap[-1][1] == elem_size
    _in_ap = eng.lower_ap_dma(in_ap, for_custom_bir_dma=True)
    _idxs_ap = eng.lower_ap(idxs_ap)
    _out_ap = eng.lower_ap(out_ap)
    return eng.add_instruction(
        mybir.InstDMAGatherAnt(
            name=nc.get_next_instruction_name(),
            ins=[*_in_ap, _idxs_ap, eng.lower_val_access(eng.to_reg(num_idxs))],
            outs=[_out_ap],
            transpose=False,
            num_idxs=num_idxs,
            elem_size=elem_size,
            stride_bytes_256=stride_bytes_256,
            gen_mode=0,
            single_packet=False,
            queue_num=queue_num,
            sbuf_tokens_per_rank=0,
            sbuf_free_dim_per_rank=0,
            sbuf_free_dim_pad_per_rank=0,
            sbuf_byte_offset=0,
        )
    )


def _build_program(fullscale: float, scale: float, Jb: tuple):
    key = (fullscale, scale, Jb)
    if key in _CACHE:
        return _CACHE[key]

    fs = float(fullscale)
    sc = float(scale)
    f32 = mybir.dt.float32
    f16 = mybir.dt.float16
    Jtot = int(sum(Jb))

    nc = bacc.Bacc(
        "TRN2", target_bir_lowering=False, debug=False, num_swdge_queues=NQ
    )
    table_d = nc.dram_tensor("table", (N, EROW), f16, kind="ExternalInput")
    idx_d = nc.dram_tensor("idxs", (P, 8 * Jtot), mybir.dt.int16, kind="ExternalInput")
    jit_d = nc.dram_tensor("jit", (2, 3), f32, kind="ExternalInput")
    corr_d = nc.dram_tensor("corr", (P, 3), f32, kind="ExternalInput")
    outh_d = nc.dram_tensor("outh", (P * Jtot, ROW), f16, kind="ExternalOutput")
    outc_d = nc.dram_tensor("outc", (P * Jtot, 3), f32, kind="ExternalOutput")

    with tile.TileContext(nc) as tc:
        with (
            tc.tile_pool(name="big", bufs=1) as big,
            tc.tile_pool(name="dst", bufs=16) as dstp,
            tc.tile_pool(name="small", bufs=1) as small,
        ):
            idx_t = big.tile([P, 8 * Jtot], mybir.dt.int16)
            nc.sync.dma_start(out=idx_t[:], in_=idx_d.ap())
            jit_t = small.tile([P, 6], f32)
            jsrc = jit_d.ap().rearrange("a b -> (a b)")
            nc.gpsimd.dma_start(
                out=jit_t[:],
                in_=bass.AP(tensor=jsrc.tensor, offset=jsrc.offset, ap=[[0, P]] + jsrc.ap),
            )
            corr_t = small.tile([P, 3], f32)
            nc.sync.dma_start(out=corr_t[:], in_=corr_d.ap())

            # coords kept fp16 and compacted by SBUF->SBUF DMA; stats and
            # transform read it with small (6B) strides, which is cheap
            ccraw = big.tile([P, Jtot, 3], f16)
            ccout = big.tile([P, Jtot, 3], f32)

            outh_3d = outh_d.ap().rearrange("(p a) c -> p a c", p=P)

            # --- banked gathers, round-robin over the 4 SWDGE queues ---
            call_i = 0
            goff = 0
            tab_ap = table_d.ap()
            for b in range(BANKS):
                base = b << BSHIFT
                bank_ap = tab_ap[base : base + (1 << BSHIFT), 0:ROW]
                done = 0
                while done < Jb[b]:
                    span = min(SPAN, Jb[b] - done)
                    ni = span * P
                    dst = dstp.tile([P, span, ROW], f16)
                    _dma_gather_raw(
                        nc,
                        dst[:],
                        bank_ap,
                        idx_t[:, 8 * goff : 8 * (goff + span)],
                        ni,
                        ROW,
                        EROW,
                        queue_num=call_i % NQ,
                    )
                    # full rows stream out in slot order (contiguous descs)
                    nc.sync.dma_start(
                        out=outh_3d[:, goff : goff + span, :],
                        in_=dst[:],
                    )
                    # compact the coord columns (fp16 SBUF->SBUF on the other
                    # HWDGE engine; a DVE strided cast here is ~23us/call)
                    nc.scalar.dma_start(
                        out=ccraw[:, goff : goff + span, :],
                        in_=dst[:, :, C : C + 3],
                    )
                    goff += span
                    done += span
                    call_i += 1

            # --- chunked stats over the slot axis ---
            SCH = 512
            nch = (Jtot + SCH - 1) // SCH
            stp = small.tile([P, 9, nch], f32)
            for j in range(nch):
                lo = j * SCH
                hi = min(Jtot, lo + SCH)
                blk = ccraw[:, lo:hi, :]
                for c in range(3):
                    nc.vector.reduce_sum(
                        out=stp[:, c, j : j + 1], in_=blk[:, :, c],
                        axis=mybir.AxisListType.X,
                    )
                    nc.vector.tensor_reduce(
                        out=stp[:, 3 + c, j : j + 1], in_=blk[:, :, c],
                        axis=mybir.AxisListType.X, op=mybir.AluOpType.min,
                    )
                    nc.vector.reduce_max(
                        out=stp[:, 6 + c, j : j + 1], in_=blk[:, :, c],
                        axis=mybir.AxisListType.X,
                    )
            st = small.tile([P, 16], f32)
            for c in range(3):
                nc.vector.reduce_sum(
                    out=st[:, c : c + 1], in_=stp[:, c, :], axis=mybir.AxisListType.X
                )
                nc.vector.tensor_reduce(
                    out=st[:, 3 + c : 4 + c], in_=stp[:, 3 + c, :],
                    axis=mybir.AxisListType.X, op=mybir.AluOpType.min,
                )
                nc.vector.reduce_max(
                    out=st[:, 6 + c : 7 + c], in_=stp[:, 6 + c, :],
                    axis=mybir.AxisListType.X,
                )
            # subtract the duplicate-padding coordinate sum
            nc.vector.tensor_tensor(
                out=st[:, 0:3], in0=st[:, 0:3], in1=corr_t[:, 0:3],
                op=mybir.AluOpType.subtract,
            )

            # --- per-cluster params (all [P, small] on DVE) ---
            pr = small.tile([P, 24], f32)
            CMEAN, CMIN, CMAX, WD, MN, MX = (
                slice(0, 3), slice(3, 6), slice(6, 9), slice(9, 12), slice(12, 15),
                slice(15, 18),
            )
            sc_t = small.tile([P, 4], f32)
            # cmean = (sum - corr) / PTS  (power of two -> exact)
            nc.vector.tensor_scalar_mul(pr[:, CMEAN], st[:, 0:3], 1.0 / PTS)
            nc.vector.tensor_tensor(
                out=pr[:, CMIN], in0=st[:, 3:6], in1=pr[:, CMEAN],
                op=mybir.AluOpType.subtract,
            )
            nc.vector.tensor_tensor(
                out=pr[:, CMAX], in0=st[:, 6:9], in1=pr[:, CMEAN],
                op=mybir.AluOpType.subtract,
            )
            nc.vector.tensor_tensor(
                out=pr[:, WD], in0=pr[:, CMAX], in1=pr[:, CMIN],
                op=mybir.AluOpType.subtract,
            )
            nc.vector.reduce_max(out=sc_t[:, 0:1], in_=pr[:, WD], axis=mybir.AxisListType.X)
            # DVE divide doesn't lower, so use IEEE reciprocal then multiply
            nc.vector.reciprocal(out=sc_t[:, 2:3], in_=sc_t[:, 0:1])
            nc.vector.tensor_scalar(
                out=sc_t[:, 3:4], in0=sc_t[:, 2:3], scalar1=fs, scalar2=-0.01,
                op0=mybir.AluOpType.mult, op1=mybir.AluOpType.add,
            )
            nc.vector.tensor_scalar(
                out=sc_t[:, 3:4], in0=sc_t[:, 3:4], scalar1=sc, scalar2=None,
                op0=mybir.AluOpType.min,
            )
            s_ap = sc_t[:, 3:4]
            nc.vector.tensor_scalar(
                out=pr[:, MN], in0=pr[:, CMIN], scalar1=s_ap, scalar2=None,
                op0=mybir.AluOpType.mult,
            )
            nc.vector.tensor_scalar(
                out=pr[:, MX], in0=pr[:, CMAX], scalar1=s_ap, scalar2=None,
                op0=mybir.AluOpType.mult,
            )
            rng_t = small.tile([P, 12], f32)
            nc.vector.tensor_tensor(
                out=rng_t[:, 0:3], in0=pr[:, MX], in1=pr[:, MN],
                op=mybir.AluOpType.subtract,
            )
            # t = fs - rng ; t0 = max(t - .001, 0) ; t1 = min(t + .001, 0)
            nc.vector.tensor_scalar(
                out=rng_t[:, 3:6], in0=rng_t[:, 0:3], scalar1=-1.0, scalar2=fs,
                op0=mybir.AluOpType.mult, op1=mybir.AluOpType.add,
            )
            nc.vector.tensor_scalar(
                out=rng_t[:, 6:9], in0=rng_t[:, 3:6], scalar1=-0.001, scalar2=0.0,
                op0=mybir.AluOpType.add, op1=mybir.AluOpType.max,
            )
            nc.vector.tensor_scalar(
                out=rng_t[:, 9:12], in0=rng_t[:, 3:6], scalar1=0.001, scalar2=0.0,
                op0=mybir.AluOpType.add, op1=mybir.AluOpType.min,
            )
            # off = (t0*j0 - mn) + t1*j1
            off_t = small.tile([P, 9], f32)
            nc.vector.tensor_tensor(
                out=off_t[:, 0:3], in0=rng_t[:, 6:9], in1=jit_t[:, 0:3],
                op=mybir.AluOpType.mult,
            )
            nc.vector.tensor_tensor(
                out=off_t[:, 3:6], in0=rng_t[:, 9:12], in1=jit_t[:, 3:6],
                op=mybir.AluOpType.mult,
            )
            nc.vector.tensor_tensor(
                out=off_t[:, 0:3], in0=off_t[:, 0:3], in1=pr[:, MN],
                op=mybir.AluOpType.subtract,
            )
            nc.vector.tensor_tensor(
                out=off_t[:, 0:3], in0=off_t[:, 0:3], in1=off_t[:, 3:6],
                op=mybir.AluOpType.add,
            )

            # --- transform: ccout = (ccraw - cmean) * s + off ---
            for c in range(3):
                nc.vector.tensor_scalar(
                    out=ccout[:, :, c], in0=ccraw[:, :, c],
                    scalar1=pr[:, c : c + 1], scalar2=s_ap,
                    op0=mybir.AluOpType.subtract, op1=mybir.AluOpType.mult,
                )
                nc.vector.tensor_scalar(
                    out=ccout[:, :, c], in0=ccout[:, :, c],
                    scalar1=off_t[:, c : c + 1], scalar2=None,
                    op0=mybir.AluOpType.add,
                )

            outc_3d = outc_d.ap().rearrange("(p a) c -> p a c", p=P)
            nc.sync.dma_start(out=outc_3d[:], in_=ccout[:])

    nc.compile()
    _CACHE[key] = nc
    return nc


def _plan(pid_full):
    """Per-core bank plan: sort each cluster's points by bank, pad cells.

    Returns (Jb tuple, per-core idx arrays [P, 8*Jtot] i16,
    per-core slot-of-original [P, PTS] i32, per-core dup (rows, counts))."""
    import hashlib

    key = hashlib.sha256(pid_full.tobytes()).hexdigest()
    if key in _PLAN_CACHE:
        return _PLAN_CACHE[key]

    MASK = (1 << BSHIFT) - 1
    per_core = []
    counts_all = []
    for k in range(NCORES):
        pk = pid_full[k * PPC : (k + 1) * PPC].reshape(P, PTS).astype(np.int64)
        bank = (pk >> BSHIFT).astype(np.int32)
        order = np.argsort(bank, axis=1, kind="stable")        # (P, PTS)
        sb = np.take_along_axis(bank, order, axis=1)
        sl = (np.take_along_axis(pk, order, axis=1) & MASK).astype(np.int32)
        cnt = np.zeros((P, BANKS), np.int32)
        for p in range(P):
            cnt[p] = np.bincount(sb[p], minlength=BANKS)
        counts_all.append(cnt)
        per_core.append((pk, order, sb, sl, cnt))

    cnt_max = np.max(np.stack([c for c in counts_all]), axis=(0, 1))  # (BANKS,)
    if np.any(np.min(np.stack(counts_all), axis=(0, 1)) == 0):
        _PLAN_CACHE[key] = None
        return None
    Jb = tuple(int(x) for x in cnt_max)
    Jtot = int(sum(Jb))
    offs = np.concatenate([[0], np.cumsum(cnt_max)]).astype(np.int64)  # (BANKS+1,)

    plans = []
    for k in range(NCORES):
        pk, order, sb, sl, cnt = per_core[k]
        idx_mat = np.zeros((P, Jtot), np.int16)
        slot_of = np.zeros((P, PTS), np.int32)
        dup_rows = []
        dup_cnts = []
        grp_start = np.zeros((P, BANKS), np.int64)
        for p in range(P):
            grp_start[p] = np.concatenate([[0], np.cumsum(cnt[p])[:-1]])
        # slot of sorted rank r: offs[bank] + (r - grp_start[bank])
        r = np.arange(PTS)[None, :]
        slot_sorted = offs[sb] + (r - np.take_along_axis(grp_start, sb, axis=1))
        np.put_along_axis(slot_of, order, slot_sorted.astype(np.int32), axis=1)
        # fill idx_mat: real entries then duplicate the cell's first entry
        pidx = np.arange(P)[:, None]
        idx_mat[pidx, slot_sorted] = sl.astype(np.int16)
        first = np.take_along_axis(sl, grp_start.astype(np.int64), axis=1)  # (P, BANKS)
        for b in range(BANKS):
            nb = cnt[:, b]
            pad = cnt_max[b] - nb                      # (P,)
            if pad.max() == 0:
                continue
            # columns offs[b]+nb ... offs[b]+Jb[b]-1 get first[:, b]
            width = int(cnt_max[b])
            cols = offs[b] + np.arange(width)[None, :]            # (1, width)
            padmask = np.arange(width)[None, :] >= nb[:, None]    # (P, width)
            vals = np.where(padmask, first[:, b : b + 1], idx_mat[pidx, cols])
            idx_mat[pidx, cols] = vals.astype(np.int16)
            rows_global = (b << BSHIFT) + first[:, b].astype(np.int64)  # (P,)
            dup_rows.append(rows_global)
            dup_cnts.append(pad)
        # wrapped int16 layout per SPAN-chunk call
        wrapped = np.zeros((P, 8 * Jtot), np.int16)
        goff = 0
        for b in range(BANKS):
            done = 0
            while done < Jb[b]:
                span = min(SPAN, Jb[b] - done)
                iv = idx_mat[:, goff : goff + span].T.ravel()     # (span*P,)
                w = iv.reshape(span * 8, 16).T                    # (16, span*8)
                wrapped[:, 8 * goff : 8 * (goff + span)] = np.tile(w, (8, 1))
                goff += span
                done += span
        plans.append(
            {
                "idx": np.ascontiguousarray(wrapped),
                "slot_of": slot_of,
                "dup_rows": np.stack(dup_rows, 1) if dup_rows else np.zeros((P, 0), np.int64),
                "dup_cnts": np.stack(dup_cnts, 1) if dup_cnts else np.zeros((P, 0), np.int64),
            }
        )
    out = (Jb, plans)
    _PLAN_CACHE[key] = out
    return out


def _reference_numpy(clusters_idx, clusters_offset, feats, coords, jitter, fullscale, scale):
    seg = clusters_idx[:, 0].astype(np.int64)
    pid = clusters_idx[:, 1].astype(np.int64)
    nC = clusters_offset.shape[0] - 1
    fs = np.float32(fullscale)
    cf = feats[pid]
    cc = coords[pid].astype(np.float32)
    cnt = np.diff(clusters_offset).astype(np.float32)[:, None]
    sums = np.zeros((nC, 3), np.float32)
    np.add.at(sums, seg, cc)
    cmean = sums / np.maximum(cnt, 1.0)
    ccc = cc - cmean[seg]
    cmin = np.full((nC, 3), np.inf, np.float32)
    cmax = np.full((nC, 3), -np.inf, np.float32)
    np.minimum.at(cmin, seg, ccc)
    np.maximum.at(cmax, seg, ccc)
    cscale = 1.0 / ((cmax - cmin) / fs).max(axis=1) - np.float32(0.01)
    cscale = np.minimum(cscale, np.float32(scale)).astype(np.float32)
    mn = cmin * cscale[:, None]
    mx = cmax * cscale[:, None]
    ccc = ccc * cscale[seg][:, None]
    rng = mx - mn
    off = (-mn + np.maximum(fs - rng - 0.001, 0.0) * jitter[0]
           + np.minimum(fs - rng + 0.001, 0.0) * jitter[1]).astype(np.float32)
    ccc = ccc + off[seg]
    return np.concatenate([cf, ccc], axis=1).astype(np.float32)


def kernel(clusters_idx, clusters_offset, feats, coords, jitter, fullscale, scale):
    clusters_idx = np.asarray(clusters_idx)
    clusters_offset = np.asarray(clusters_offset)
    feats = np.asarray(feats, dtype=np.float32)
    coords = np.asarray(coords, dtype=np.float32)
    jitter = np.asarray(jitter, dtype=np.float32)

    fs = float(np.asarray(fullscale).item()) if not isinstance(fullscale, (int, float)) else float(fullscale)
    sc = float(np.asarray(scale).item()) if not isinstance(scale, (int, float)) else float(scale)

    uniform = (
        clusters_idx.shape == (S, 2)
        and clusters_offset.shape == (NCLUSTER + 1,)
        and feats.shape == (N, C)
        and coords.shape == (N, 3)
        and np.array_equal(
            clusters_offset,
            np.arange(NCLUSTER + 1, dtype=np.int64) * PTS,
        )
        and np.array_equal(
            clusters_idx[:, 0],
            np.repeat(np.arange(NCLUSTER, dtype=np.int64), PTS),
        )
    )
    if not uniform:
        return _reference_numpy(
            clusters_idx, clusters_offset, feats, coords, jitter, fs, sc
        )

    pid_full = np.ascontiguousarray(clusters_idx[:, 1].astype(np.int32))
    plan = _plan(pid_full)
    if plan is None:
        return _reference_numpy(
            clusters_idx, clusters_offset, feats, coords, jitter, fs, sc
        )
    Jb, plans = plan
    Jtot = int(sum(Jb))

    nc = _build_program(fs, sc, Jb)

    table = np.zeros((N, EROW), dtype=np.float16)
    table[:, :C] = feats
    table[:, C : C + 3] = coords
    coords16 = table[:, C : C + 3].astype(np.float32)

    in_maps = []
    for k in range(NCORES):
        pl = plans[k]
        corr = (
            coords16[pl["dup_rows"]] * pl["dup_cnts"][:, :, None]
        ).sum(axis=1).astype(np.float32)
        in_maps.append(
            {"table": table, "idxs": pl["idx"], "jit": jitter, "corr": corr}
        )

    res = bass_utils.run_bass_kernel_spmd(
        nc, in_maps, core_ids=list(range(NCORES)), trace=TRACE
    )
    global LAST_RESULTS
    LAST_RESULTS = res

    out = np.empty((S, ROW), dtype=np.float32)
    pidx = np.arange(P)[:, None]
    for k in range(NCORES):
        sl = slice(k * PPC, (k + 1) * PPC)
        slot_of = plans[k]["slot_of"]
        oh = res.results[k]["outh"].reshape(P, Jtot, ROW)
        oc = res.results[k]["outc"].reshape(P, Jtot, 3)
        out[sl, :C] = oh[pidx, slot_of, :C].reshape(PPC, C).astype(np.float32)
        out[sl, C:] = oc[pidx, slot_of].reshape(PPC, 3)
    return out


# revision 15
# speedup vs baseline: 1.4889x; 1.4889x over previous
"""PointGroup clusters_voxelization kernel for Trainium2 (8 NeuronCores).

Strategy (sharding_hint): shard the 1024 clusters across 8 cores, 128
clusters each; feats/coords replicated. On each core, cluster c maps to
SBUF partition c, so all segment reductions (sum/min/max over the 2048
points of a cluster) are single-partition free-axis reductions.

v4 data movement (per core): the naive path (one indirect DMA per point
slot, 2048 calls) is bound by SWDGE descriptor generation on the Pool
engine (~1.4us/call -> ~2.9ms). Instead we use the batched dma_gather
custom instruction, which is ~9ns/index on one Q7 pair and scales ~3x
across the 4 SWDGE queues (one Q7 core pair each):

  - host builds a (N, 128) fp16 table (feats||coords padded to a 256B
    row, dma_gather requires 256B-aligned elements; fp16 costs ~3e-4
    rel err, budget is 2e-2)
  - dma_gather indices are int16 (<32768), so the table is processed as
    32 banks of 32768 rows; host re-sorts each cluster's points by bank
    and pads each (cluster, bank) cell to the bank-wide max count with
    duplicates of an existing member (min/max unaffected; the duplicate
    coordinate sum is passed in as a per-cluster correction so the mean
    stays exact)
  - gather calls (one per bank x 32-slot chunk, NI=4096 idx) round-robin
    the 4 SWDGE queues; gathered rows land in "slot" order
  - full rows stream out to a slot-ordered fp16 tensor; coords are cast
    to f32 on-chip; stats/transform as before; transformed coords out as
    a slot-ordered f32 tensor
  - host applies the inverse slot permutation per cluster and assembles
    feats(fp16->f32) || coords(f32)

The per-bank pad counts (Jb) are data-dependent and baked into the
program; the compiled program is cached keyed on them (one compile per
distinct input distribution; the numpy fallback covers degenerate data).
"""
import numpy as np

import concourse.bass as bass
import concourse.bacc as bacc
import concourse.tile as tile
import concourse.mybir as mybir
from concourse import bass_utils

N = 1048576
C = 32
NCLUSTER = 1024
PTS = 2048
S = NCLUSTER * PTS
NCORES = 8
P = 128                      # partitions = clusters per core
PPC = S // NCORES            # points per core = 262144
ROW = C + 3                  # 35 real elements per row
EROW = 128                   # padded fp16 row (256B) for dma_gather
BANKS = 32
BSHIFT = 15                  # 32768 rows per bank
SPAN = 32                    # slots per gather call (NI = SPAN*128 = 4096)
NQ = 4                       # SWDGE queues (Q7 core pairs)

_CACHE = {}
_PLAN_CACHE = {}

# set by kernel_timing.profile() to capture an NTFF trace on the next run
TRACE = False
LAST_RESULTS = None


def _dma_gather_raw(nc, out_ap, in_ap, idxs_ap, num_idxs, elem_size, elem_step, queue_num):
    """bass.dma_gather without the elem_size%256 assert.

    The Q7 ucode only requires the row STRIDE to be a multiple of 256B
    (address math is idx * stride_bytes_256 * 256); the payload length is
    free for non-transpose gathers, so we fetch just the 35 real fp16
    elements (70B) of each 256B-aligned table row."""
    eng = nc.gpsimd
    stride_bytes = elem_step * mybir.dt.size(in_ap.dtype)
    stride_bytes_256 = stride_bytes // 256
    assert stride_bytes % 256 == 0 and stride_bytes_256 < 256
    assert in_ap.ap[0][0] == elem_step
    assert in_ap.ap[-1][1] == elem_size
    assert out_ap.ap[-1][1] == elem_size
    _in_ap = eng.lower_ap_dma(in_ap, for_custom_bir_dma=True)
    _idxs_ap = eng.lower_ap(idxs_ap)
    _out_ap = eng.lower_ap(out_ap)
    return eng.add_instruction(
        mybir.InstDMAGatherAnt(
            name=nc.get_next_instruction_name(),
            ins=[*_in_ap, _idxs_ap, eng.lower_val_access(eng.to_reg(num_idxs))],
            outs=[_out_ap],
            transpose=False,
            num_idxs=num_idxs,
            elem_size=elem_size,
            stride_bytes_256=stride_bytes_256,
            gen_mode=0,
            single_packet=False,
            queue_num=queue_num,
            sbuf_tokens_per_rank=0,
            sbuf_free_dim_per_rank=0,
            sbuf_free_dim_pad_per_rank=0,
            sbuf_byte_offset=0,
        )
    )


def _build_program(fullscale: float, scale: float, Jb: tuple):
    key = (fullscale, scale, Jb)
    if key in _CACHE:
        return _CACHE[key]

    fs = float(fullscale)
    sc = float(scale)
    f32 = mybir.dt.float32
    f16 = mybir.dt.float16
    Jtot = int(sum(Jb))

    nc = bacc.Bacc(
        "TRN2", target_bir_lowering=False, debug=False, num_swdge_queues=NQ
    )
    table_d = nc.dram_tensor("table", (N, EROW), f16, kind="ExternalInput")
    idx_d = nc.dram_tensor("idxs", (P, 8 * Jtot), mybir.dt.int16, kind="ExternalInput")
    jit_d = nc.dram_tensor("jit", (2, 3), f32, kind="ExternalInput")
    corr_d = nc.dram_tensor("corr", (P, 3), f32, kind="ExternalInput")
    outh_d = nc.dram_tensor("outh", (P * Jtot, ROW), f16, kind="ExternalOutput")
    outc_d = nc.dram_tensor("outc", (P * Jtot, 3), f32, kind="ExternalOutput")

    with tile.TileContext(nc) as tc:
        with (
            tc.tile_pool(name="big", bufs=1) as big,
            tc.tile_pool(name="dst", bufs=16) as dstp,
            tc.tile_pool(name="small", bufs=1) as small,
        ):
            # split the idx load so the first bank's gathers start immediately
            idx_t = big.tile([P, 8 * Jtot], mybir.dt.int16)
            head = 8 * int(Jb[0])
            nc.sync.dma_start(out=idx_t[:, 0:head], in_=idx_d.ap()[:, 0:head])
            nc.sync.dma_start(
                out=idx_t[:, head : 8 * Jtot], in_=idx_d.ap()[:, head : 8 * Jtot]
            )
            jit_t = small.tile([P, 6], f32)
            jsrc = jit_d.ap().rearrange("a b -> (a b)")
            nc.gpsimd.dma_start(
                out=jit_t[:],
                in_=bass.AP(tensor=jsrc.tensor, offset=jsrc.offset, ap=[[0, P]] + jsrc.ap),
            )
            corr_t = small.tile([P, 3], f32)
            nc.sync.dma_start(out=corr_t[:], in_=corr_d.ap())

            # coords kept fp16 and compacted by SBUF->SBUF DMA; stats and
            # transform read it with small (6B) strides, which is cheap
            ccraw = big.tile([P, Jtot, 3], f16)
            ccout = big.tile([P, Jtot, 3], f32)

            outh_3d = outh_d.ap().rearrange("(p a) c -> p a c", p=P)

            # --- banked gathers, round-robin over the 4 SWDGE queues ---
            call_i = 0
            goff = 0
            tab_ap = table_d.ap()
            for b in range(BANKS):
                base = b << BSHIFT
                bank_ap = tab_ap[base : base + (1 << BSHIFT), 0:ROW]
                done = 0
                while done < Jb[b]:
                    span = min(SPAN, Jb[b] - done)
                    ni = span * P
                    dst = dstp.tile([P, span, ROW], f16)
                    _dma_gather_raw(
                        nc,
                        dst[:],
                        bank_ap,
                        idx_t[:, 8 * goff : 8 * (goff + span)],
                        ni,
                        ROW,
                        EROW,
                        queue_num=call_i % NQ,
                    )
                    # full rows stream out in slot order (contiguous descs)
                    nc.sync.dma_start(
                        out=outh_3d[:, goff : goff + span, :],
                        in_=dst[:],
                    )
                    # compact the coord columns (fp16 SBUF->SBUF on the other
                    # HWDGE engine; a DVE strided cast here is ~23us/call)
                    nc.scalar.dma_start(
                        out=ccraw[:, goff : goff + span, :],
                        in_=dst[:, :, C : C + 3],
                    )
                    goff += span
                    done += span
                    call_i += 1

            # --- chunked stats over the slot axis ---
            SCH = 512
            nch = (Jtot + SCH - 1) // SCH
            stp = small.tile([P, 9, nch], f32)
            for j in range(nch):
                lo = j * SCH
                hi = min(Jtot, lo + SCH)
                blk = ccraw[:, lo:hi, :]
                for c in range(3):
                    nc.vector.reduce_sum(
                        out=stp[:, c, j : j + 1], in_=blk[:, :, c],
                        axis=mybir.AxisListType.X,
                    )
                    nc.vector.tensor_reduce(
                        out=stp[:, 3 + c, j : j + 1], in_=blk[:, :, c],
                        axis=mybir.AxisListType.X, op=mybir.AluOpType.min,
                    )
                    nc.vector.reduce_max(
                        out=stp[:, 6 + c, j : j + 1], in_=blk[:, :, c],
                        axis=mybir.AxisListType.X,
                    )
            st = small.tile([P, 16], f32)
            for c in range(3):
                nc.vector.reduce_sum(
                    out=st[:, c : c + 1], in_=stp[:, c, :], axis=mybir.AxisListType.X
                )
                nc.vector.tensor_reduce(
                    out=st[:, 3 + c : 4 + c], in_=stp[:, 3 + c, :],
                    axis=mybir.AxisListType.X, op=mybir.AluOpType.min,
                )
                nc.vector.reduce_max(
                    out=st[:, 6 + c : 7 + c], in_=stp[:, 6 + c, :],
                    axis=mybir.AxisListType.X,
                )
            # subtract the duplicate-padding coordinate sum
            nc.vector.tensor_tensor(
                out=st[:, 0:3], in0=st[:, 0:3], in1=corr_t[:, 0:3],
                op=mybir.AluOpType.subtract,
            )

            # --- per-cluster params (all [P, small] on DVE) ---
            pr = small.tile([P, 24], f32)
            CMEAN, CMIN, CMAX, WD, MN, MX = (
                slice(0, 3), slice(3, 6), slice(6, 9), slice(9, 12), slice(12, 15),
                slice(15, 18),
            )
            sc_t = small.tile([P, 4], f32)
            # cmean = (sum - corr) / PTS  (power of two -> exact)
            nc.vector.tensor_scalar_mul(pr[:, CMEAN], st[:, 0:3], 1.0 / PTS)
            nc.vector.tensor_tensor(
                out=pr[:, CMIN], in0=st[:, 3:6], in1=pr[:, CMEAN],
                op=mybir.AluOpType.subtract,
            )
            nc.vector.tensor_tensor(
                out=pr[:, CMAX], in0=st[:, 6:9], in1=pr[:, CMEAN],
                op=mybir.AluOpType.subtract,
            )
            nc.vector.tensor_tensor(
                out=pr[:, WD], in0=pr[:, CMAX], in1=pr[:, CMIN],
                op=mybir.AluOpType.subtract,
            )
            nc.vector.reduce_max(out=sc_t[:, 0:1], in_=pr[:, WD], axis=mybir.AxisListType.X)
            # DVE divide doesn't lower, so use IEEE reciprocal then multiply
            nc.vector.reciprocal(out=sc_t[:, 2:3], in_=sc_t[:, 0:1])
            nc.vector.tensor_scalar(
                out=sc_t[:, 3:4], in0=sc_t[:, 2:3], scalar1=fs, scalar2=-0.01,
                op0=mybir.AluOpType.mult, op1=mybir.AluOpType.add,
            )
            nc.vector.tensor_scalar(
                out=sc_t[:, 3:4], in0=sc_t[:, 3:4], scalar1=sc, scalar2=None,
                op0=mybir.AluOpType.min,
            )
            s_ap = sc_t[:, 3:4]
            nc.vector.tensor_scalar(
                out=pr[:, MN], in0=pr[:, CMIN], scalar1=s_ap, scalar2=None,
                op0=mybir.AluOpType.mult,
            )
            nc.vector.tensor_scalar(
                out=pr[:, MX], in0=pr[:, CMAX], scalar1=s_ap, scalar2=None,
                op0=mybir.AluOpType.mult,
            )
            rng_t = small.tile([P, 12], f32)
            nc.vector.tensor_tensor(
                out=rng_t[:, 0:3], in0=pr[:, MX], in1=pr[:, MN],
                op=mybir.AluOpType.subtract,
            )
            # t = fs - rng ; t0 = max(t - .001, 0) ; t1 = min(t + .001, 0)
            nc.vector.tensor_scalar(
                out=rng_t[:, 3:6], in0=rng_t[:, 0:3], scalar1=-1.0, scalar2=fs,
                op0=mybir.AluOpType.mult, op1=mybir.AluOpType.add,
            )
            nc.vector.tensor_scalar(
                out=rng_t[:, 6:9], in0=rng_t[:, 3:6], scalar1=-0.001, scalar2=0.0,
                op0=mybir.AluOpType.add, op1=mybir.AluOpType.max,
            )
            nc.vector.tensor_scalar(
                out=rng_t[:, 9:12], in0=rng_t[:, 3:6], scalar1=0.001, scalar2=0.0,
                op0=mybir.AluOpType.add, op1=mybir.AluOpType.min,
            )
            # off = (t0*j0 - mn) + t1*j1
            off_t = small.tile([P, 9], f32)
            nc.vector.tensor_tensor(
                out=off_t[:, 0:3], in0=rng_t[:, 6:9], in1=jit_t[:, 0:3],
                op=mybir.AluOpType.mult,
            )
            nc.vector.tensor_tensor(
                out=off_t[:, 3:6], in0=rng_t[:, 9:12], in1=jit_t[:, 3:6],
                op=mybir.AluOpType.mult,
            )
            nc.vector.tensor_tensor(
                out=off_t[:, 0:3], in0=off_t[:, 0:3], in1=pr[:, MN],
                op=mybir.AluOpType.subtract,
            )
            nc.vector.tensor_tensor(
                out=off_t[:, 0:3], in0=off_t[:, 0:3], in1=off_t[:, 3:6],
                op=mybir.AluOpType.add,
            )

            # --- transform: ccout = (ccraw - cmean) * s + off ---
            for c in range(3):
                nc.vector.tensor_scalar(
                    out=ccout[:, :, c], in0=ccraw[:, :, c],
                    scalar1=pr[:, c : c + 1], scalar2=s_ap,
                    op0=mybir.AluOpType.subtract, op1=mybir.AluOpType.mult,
                )
                nc.vector.tensor_scalar(
                    out=ccout[:, :, c], in0=ccout[:, :, c],
                    scalar1=off_t[:, c : c + 1], scalar2=None,
                    op0=mybir.AluOpType.add,
                )

            outc_3d = outc_d.ap().rearrange("(p a) c -> p a c", p=P)
            nc.sync.dma_start(out=outc_3d[:], in_=ccout[:])

    nc.compile()
    _CACHE[key] = nc
    return nc


def _plan(pid_full):
    """Per-core bank plan: sort each cluster's points by bank, pad cells.

    Returns (Jb tuple, per-core idx arrays [P, 8*Jtot] i16,
    per-core slot-of-original [P, PTS] i32, per-core dup (rows, counts))."""
    import hashlib

    key = hashlib.sha256(pid_full.tobytes()).hexdigest()
    if key in _PLAN_CACHE:
        return _PLAN_CACHE[key]

    MASK = (1 << BSHIFT) - 1
    per_core = []
    counts_all = []
    for k in range(NCORES):
        pk = pid_full[k * PPC : (k + 1) * PPC].reshape(P, PTS).astype(np.int64)
        bank = (pk >> BSHIFT).astype(np.int32)
        order = np.argsort(bank, axis=1, kind="stable")        # (P, PTS)
        sb = np.take_along_axis(bank, order, axis=1)
        sl = (np.take_along_axis(pk, order, axis=1) & MASK).astype(np.int32)
        cnt = np.zeros((P, BANKS), np.int32)
        for p in range(P):
            cnt[p] = np.bincount(sb[p], minlength=BANKS)
        counts_all.append(cnt)
        per_core.append((pk, order, sb, sl, cnt))

    cnt_max = np.max(np.stack([c for c in counts_all]), axis=(0, 1))  # (BANKS,)
    if np.any(np.min(np.stack(counts_all), axis=(0, 1)) == 0):
        _PLAN_CACHE[key] = None
        return None
    Jb = tuple(int(x) for x in cnt_max)
    Jtot = int(sum(Jb))
    offs = np.concatenate([[0], np.cumsum(cnt_max)]).astype(np.int64)  # (BANKS+1,)

    plans = []
    for k in range(NCORES):
        pk, order, sb, sl, cnt = per_core[k]
        idx_mat = np.zeros((P, Jtot), np.int16)
        slot_of = np.zeros((P, PTS), np.int32)
        dup_rows = []
        dup_cnts = []
        grp_start = np.zeros((P, BANKS), np.int64)
        for p in range(P):
            grp_start[p] = np.concatenate([[0], np.cumsum(cnt[p])[:-1]])
        # slot of sorted rank r: offs[bank] + (r - grp_start[bank])
        r = np.arange(PTS)[None, :]
        slot_sorted = offs[sb] + (r - np.take_along_axis(grp_start, sb, axis=1))
        np.put_along_axis(slot_of, order, slot_sorted.astype(np.int32), axis=1)
        # fill idx_mat: real entries then duplicate the cell's first entry
        pidx = np.arange(P)[:, None]
        idx_mat[pidx, slot_sorted] = sl.astype(np.int16)
        first = np.take_along_axis(sl, grp_start.astype(np.int64), axis=1)  # (P, BANKS)
        for b in range(BANKS):
            nb = cnt[:, b]
            pad = cnt_max[b] - nb                      # (P,)
            if pad.max() == 0:
                continue
            # columns offs[b]+nb ... offs[b]+Jb[b]-1 get first[:, b]
            width = int(cnt_max[b])
            cols = offs[b] + np.arange(width)[None, :]            # (1, width)
            padmask = np.arange(width)[None, :] >= nb[:, None]    # (P, width)
            vals = np.where(padmask, first[:, b : b + 1], idx_mat[pidx, cols])
            idx_mat[pidx, cols] = vals.astype(np.int16)
            rows_global = (b << BSHIFT) + first[:, b].astype(np.int64)  # (P,)
            dup_rows.append(rows_global)
            dup_cnts.append(pad)
        # wrapped int16 layout per SPAN-chunk call
        wrapped = np.zeros((P, 8 * Jtot), np.int16)
        goff = 0
        for b in range(BANKS):
            done = 0
            while done < Jb[b]:
                span = min(SPAN, Jb[b] - done)
                iv = idx_mat[:, goff : goff + span].T.ravel()     # (span*P,)
                w = iv.reshape(span * 8, 16).T                    # (16, span*8)
                wrapped[:, 8 * goff : 8 * (goff + span)] = np.tile(w, (8, 1))
                goff += span
                done += span
        plans.append(
            {
                "idx": np.ascontiguousarray(wrapped),
                "slot_of": slot_of,
                "dup_rows": np.stack(dup_rows, 1) if dup_rows else np.zeros((P, 0), np.int64),
                "dup_cnts": np.stack(dup_cnts, 1) if dup_cnts else np.zeros((P, 0), np.int64),
            }
        )
    out = (Jb, plans)
    _PLAN_CACHE[key] = out
    return out


def _reference_numpy(clusters_idx, clusters_offset, feats, coords, jitter, fullscale, scale):
    seg = clusters_idx[:, 0].astype(np.int64)
    pid = clusters_idx[:, 1].astype(np.int64)
    nC = clusters_offset.shape[0] - 1
    fs = np.float32(fullscale)
    cf = feats[pid]
    cc = coords[pid].astype(np.float32)
    cnt = np.diff(clusters_offset).astype(np.float32)[:, None]
    sums = np.zeros((nC, 3), np.float32)
    np.add.at(sums, seg, cc)
    cmean = sums / np.maximum(cnt, 1.0)
    ccc = cc - cmean[seg]
    cmin = np.full((nC, 3), np.inf, np.float32)
    cmax = np.full((nC, 3), -np.inf, np.float32)
    np.minimum.at(cmin, seg, ccc)
    np.maximum.at(cmax, seg, ccc)
    cscale = 1.0 / ((cmax - cmin) / fs).max(axis=1) - np.float32(0.01)
    cscale = np.minimum(cscale, np.float32(scale)).astype(np.float32)
    mn = cmin * cscale[:, None]
    mx = cmax * cscale[:, None]
    ccc = ccc * cscale[seg][:, None]
    rng = mx - mn
    off = (-mn + np.maximum(fs - rng - 0.001, 0.0) * jitter[0]
           + np.minimum(fs - rng + 0.001, 0.0) * jitter[1]).astype(np.float32)
    ccc = ccc + off[seg]
    return np.concatenate([cf, ccc], axis=1).astype(np.float32)


def kernel(clusters_idx, clusters_offset, feats, coords, jitter, fullscale, scale):
    clusters_idx = np.asarray(clusters_idx)
    clusters_offset = np.asarray(clusters_offset)
    feats = np.asarray(feats, dtype=np.float32)
    coords = np.asarray(coords, dtype=np.float32)
    jitter = np.asarray(jitter, dtype=np.float32)

    fs = float(np.asarray(fullscale).item()) if not isinstance(fullscale, (int, float)) else float(fullscale)
    sc = float(np.asarray(scale).item()) if not isinstance(scale, (int, float)) else float(scale)

    uniform = (
        clusters_idx.shape == (S, 2)
        and clusters_offset.shape == (NCLUSTER + 1,)
        and feats.shape == (N, C)
        and coords.shape == (N, 3)
        and np.array_equal(
            clusters_offset,
            np.arange(NCLUSTER + 1, dtype=np.int64) * PTS,
        )
        and np.array_equal(
            clusters_idx[:, 0],
            np.repeat(np.arange(NCLUSTER, dtype=np.int64), PTS),
        )
    )
    if not uniform:
        return _reference_numpy(
            clusters_idx, clusters_offset, feats, coords, jitter, fs, sc
        )

    pid_full = np.ascontiguousarray(clusters_idx[:, 1].astype(np.int32))
    plan = _plan(pid_full)
    if plan is None:
        return _reference_numpy(
            clusters_idx, clusters_offset, feats, coords, jitter, fs, sc
        )
    Jb, plans = plan
    Jtot = int(sum(Jb))

    nc = _build_program(fs, sc, Jb)

    table = np.zeros((N, EROW), dtype=np.float16)
    table[:, :C] = feats
    table[:, C : C + 3] = coords
    coords16 = table[:, C : C + 3].astype(np.float32)

    in_maps = []
    for k in range(NCORES):
        pl = plans[k]
        corr = (
            coords16[pl["dup_rows"]] * pl["dup_cnts"][:, :, None]
        ).sum(axis=1).astype(np.float32)
        in_maps.append(
            {"table": table, "idxs": pl["idx"], "jit": jitter, "corr": corr}
        )

    res = bass_utils.run_bass_kernel_spmd(
        nc, in_maps, core_ids=list(range(NCORES)), trace=TRACE
    )
    global LAST_RESULTS
    LAST_RESULTS = res

    out = np.empty((S, ROW), dtype=np.float32)
    pidx = np.arange(P)[:, None]
    for k in range(NCORES):
        sl = slice(k * PPC, (k + 1) * PPC)
        slot_of = plans[k]["slot_of"]
        oh = res.results[k]["outh"].reshape(P, Jtot, ROW)
        oc = res.results[k]["outc"].reshape(P, Jtot, 3)
        out[sl, :C] = oh[pidx, slot_of, :C].reshape(PPC, C).astype(np.float32)
        out[sl, C:] = oc[pidx, slot_of].reshape(PPC, 3)
    return out


# revision 16
# speedup vs baseline: 1.4894x; 1.0003x over previous
"""PointGroup clusters_voxelization kernel for Trainium2 (8 NeuronCores).

Strategy (sharding_hint): shard the 1024 clusters across 8 cores, 128
clusters each; feats/coords replicated. On each core, cluster c maps to
SBUF partition c, so all segment reductions (sum/min/max over the 2048
points of a cluster) are single-partition free-axis reductions.

v4 data movement (per core): the naive path (one indirect DMA per point
slot, 2048 calls) is bound by SWDGE descriptor generation on the Pool
engine (~1.4us/call -> ~2.9ms). Instead we use the batched dma_gather
custom instruction, which is ~9ns/index on one Q7 pair and scales ~3x
across the 4 SWDGE queues (one Q7 core pair each):

  - host builds a (N, 128) fp16 table (feats||coords padded to a 256B
    row, dma_gather requires 256B-aligned elements; fp16 costs ~3e-4
    rel err, budget is 2e-2)
  - dma_gather indices are int16 (<32768), so the table is processed as
    32 banks of 32768 rows; host re-sorts each cluster's points by bank
    and pads each (cluster, bank) cell to the bank-wide max count with
    duplicates of an existing member (min/max unaffected; the duplicate
    coordinate sum is passed in as a per-cluster correction so the mean
    stays exact)
  - gather calls (one per bank x 32-slot chunk, NI=4096 idx) round-robin
    the 4 SWDGE queues; gathered rows land in "slot" order
  - full rows stream out to a slot-ordered fp16 tensor; coords are cast
    to f32 on-chip; stats/transform as before; transformed coords out as
    a slot-ordered f32 tensor
  - host applies the inverse slot permutation per cluster and assembles
    feats(fp16->f32) || coords(f32)

The per-bank pad counts (Jb) are data-dependent and baked into the
program; the compiled program is cached keyed on them (one compile per
distinct input distribution; the numpy fallback covers degenerate data).
"""
import numpy as np

import concourse.bass as bass
import concourse.bacc as bacc
import concourse.tile as tile
import concourse.mybir as mybir
from concourse import bass_utils

N = 1048576
C = 32
NCLUSTER = 1024
PTS = 2048
S = NCLUSTER * PTS
NCORES = 8
P = 128                      # partitions = clusters per core
PPC = S // NCORES            # points per core = 262144
ROW = C + 3                  # 35 real elements per row
EROW = 128                   # padded fp16 row (256B) for dma_gather
BANKS = 32
BSHIFT = 15                  # 32768 rows per bank
SPAN = 32                    # slots per gather call (NI = SPAN*128 = 4096)
NQ = 4                       # SWDGE queues (Q7 core pairs)

_CACHE = {}
_PLAN_CACHE = {}

# set by kernel_timing.profile() to capture an NTFF trace on the next run
TRACE = False
LAST_RESULTS = None


def _dma_gather_raw(nc, out_ap, in_ap, idxs_ap, num_idxs, elem_size, elem_step, queue_num):
    """bass.dma_gather without the elem_size%256 assert.

    The Q7 ucode only requires the row STRIDE to be a multiple of 256B
    (address math is idx * stride_bytes_256 * 256); the payload length is
    free for non-transpose gathers, so we fetch just the 35 real fp16
    elements (70B) of each 256B-aligned table row."""
    eng = nc.gpsimd
    stride_bytes = elem_step * mybir.dt.size(in_ap.dtype)
    stride_bytes_256 = stride_bytes // 256
    assert stride_bytes % 256 == 0 and stride_bytes_256 < 256
    assert in_ap.ap[0][0] == elem_step
    assert in_ap.ap[-1][1] == elem_size
    assert out_ap.ap[-1][1] == elem_size
    _in_ap = eng.lower_ap_dma(in_ap, for_custom_bir_dma=True)
    _idxs_ap = eng.lower_ap(idxs_ap)
    _out_ap = eng.lower_ap(out_ap)
    return eng.add_instruction(
        mybir.InstDMAGatherAnt(
            name=nc.get_next_instruction_name(),
            ins=[*_in_ap, _idxs_ap, eng.lower_val_access(eng.to_reg(num_idxs))],
            outs=[_out_ap],
            transpose=False,
            num_idxs=num_idxs,
            elem_size=elem_size,
            stride_bytes_256=stride_bytes_256,
            gen_mode=0,
            single_packet=False,
            queue_num=queue_num,
            sbuf_tokens_per_rank=0,
            sbuf_free_dim_per_rank=0,
            sbuf_free_dim_pad_per_rank=0,
            sbuf_byte_offset=0,
        )
    )


def _build_program(fullscale: float, scale: float, Jb: tuple):
    key = (fullscale, scale, Jb)
    if key in _CACHE:
        return _CACHE[key]

    fs = float(fullscale)
    sc = float(scale)
    f32 = mybir.dt.float32
    f16 = mybir.dt.float16
    Jtot = int(sum(Jb))

    nc = bacc.Bacc(
        "TRN2", target_bir_lowering=False, debug=False, num_swdge_queues=NQ
    )
    table_d = nc.dram_tensor("table", (N, EROW), f16, kind="ExternalInput")
    idx_d = nc.dram_tensor("idxs", (P, 8 * Jtot), mybir.dt.int16, kind="ExternalInput")
    jit_d = nc.dram_tensor("jit", (2, 3), f32, kind="ExternalInput")
    corr_d = nc.dram_tensor("corr", (P, 3), f32, kind="ExternalInput")
    outh_d = nc.dram_tensor("outh", (P * Jtot, ROW), f16, kind="ExternalOutput")
    outc_d = nc.dram_tensor("outc", (P * Jtot, 3), f32, kind="ExternalOutput")

    with tile.TileContext(nc) as tc:
        with (
            tc.tile_pool(name="big", bufs=1) as big,
            tc.tile_pool(name="dst", bufs=32) as dstp,
            tc.tile_pool(name="small", bufs=1) as small,
        ):
            # split the idx load so the first bank's gathers start immediately
            idx_t = big.tile([P, 8 * Jtot], mybir.dt.int16)
            head = 8 * int(Jb[0])
            nc.sync.dma_start(out=idx_t[:, 0:head], in_=idx_d.ap()[:, 0:head])
            nc.sync.dma_start(
                out=idx_t[:, head : 8 * Jtot], in_=idx_d.ap()[:, head : 8 * Jtot]
            )
            jit_t = small.tile([P, 6], f32)
            jsrc = jit_d.ap().rearrange("a b -> (a b)")
            nc.gpsimd.dma_start(
                out=jit_t[:],
                in_=bass.AP(tensor=jsrc.tensor, offset=jsrc.offset, ap=[[0, P]] + jsrc.ap),
            )
            corr_t = small.tile([P, 3], f32)
            nc.sync.dma_start(out=corr_t[:], in_=corr_d.ap())

            # coords kept fp16 and compacted by SBUF->SBUF DMA; stats and
            # transform read it with small (6B) strides, which is cheap
            ccraw = big.tile([P, Jtot, 3], f16)
            ccout = big.tile([P, Jtot, 3], f32)

            outh_3d = outh_d.ap().rearrange("(p a) c -> p a c", p=P)

            # --- banked gathers, round-robin over the 4 SWDGE queues ---
            call_i = 0
            goff = 0
            tab_ap = table_d.ap()
            for b in range(BANKS):
                base = b << BSHIFT
                bank_ap = tab_ap[base : base + (1 << BSHIFT), 0:ROW]
                done = 0
                while done < Jb[b]:
                    span = min(SPAN, Jb[b] - done)
                    ni = span * P
                    dst = dstp.tile([P, span, ROW], f16)
                    _dma_gather_raw(
                        nc,
                        dst[:],
                        bank_ap,
                        idx_t[:, 8 * goff : 8 * (goff + span)],
                        ni,
                        ROW,
                        EROW,
                        queue_num=call_i % NQ,
                    )
                    # full rows stream out in slot order (contiguous descs)
                    nc.sync.dma_start(
                        out=outh_3d[:, goff : goff + span, :],
                        in_=dst[:],
                    )
                    # compact the coord columns (fp16 SBUF->SBUF on the other
                    # HWDGE engine; a DVE strided cast here is ~23us/call)
                    nc.scalar.dma_start(
                        out=ccraw[:, goff : goff + span, :],
                        in_=dst[:, :, C : C + 3],
                    )
                    goff += span
                    done += span
                    call_i += 1

            # --- chunked stats over the slot axis ---
            SCH = 512
            nch = (Jtot + SCH - 1) // SCH
            stp = small.tile([P, 9, nch], f32)
            for j in range(nch):
                lo = j * SCH
                hi = min(Jtot, lo + SCH)
                blk = ccraw[:, lo:hi, :]
                for c in range(3):
                    nc.vector.reduce_sum(
                        out=stp[:, c, j : j + 1], in_=blk[:, :, c],
                        axis=mybir.AxisListType.X,
                    )
                    nc.vector.tensor_reduce(
                        out=stp[:, 3 + c, j : j + 1], in_=blk[:, :, c],
                        axis=mybir.AxisListType.X, op=mybir.AluOpType.min,
                    )
                    nc.vector.reduce_max(
                        out=stp[:, 6 + c, j : j + 1], in_=blk[:, :, c],
                        axis=mybir.AxisListType.X,
                    )
            st = small.tile([P, 16], f32)
            for c in range(3):
                nc.vector.reduce_sum(
                    out=st[:, c : c + 1], in_=stp[:, c, :], axis=mybir.AxisListType.X
                )
                nc.vector.tensor_reduce(
                    out=st[:, 3 + c : 4 + c], in_=stp[:, 3 + c, :],
                    axis=mybir.AxisListType.X, op=mybir.AluOpType.min,
                )
                nc.vector.reduce_max(
                    out=st[:, 6 + c : 7 + c], in_=stp[:, 6 + c, :],
                    axis=mybir.AxisListType.X,
                )
            # subtract the duplicate-padding coordinate sum
            nc.vector.tensor_tensor(
                out=st[:, 0:3], in0=st[:, 0:3], in1=corr_t[:, 0:3],
                op=mybir.AluOpType.subtract,
            )

            # --- per-cluster params (all [P, small] on DVE) ---
            pr = small.tile([P, 24], f32)
            CMEAN, CMIN, CMAX, WD, MN, MX = (
                slice(0, 3), slice(3, 6), slice(6, 9), slice(9, 12), slice(12, 15),
                slice(15, 18),
            )
            sc_t = small.tile([P, 4], f32)
            # cmean = (sum - corr) / PTS  (power of two -> exact)
            nc.vector.tensor_scalar_mul(pr[:, CMEAN], st[:, 0:3], 1.0 / PTS)
            nc.vector.tensor_tensor(
                out=pr[:, CMIN], in0=st[:, 3:6], in1=pr[:, CMEAN],
                op=mybir.AluOpType.subtract,
            )
            nc.vector.tensor_tensor(
                out=pr[:, CMAX], in0=st[:, 6:9], in1=pr[:, CMEAN],
                op=mybir.AluOpType.subtract,
            )
            nc.vector.tensor_tensor(
                out=pr[:, WD], in0=pr[:, CMAX], in1=pr[:, CMIN],
                op=mybir.AluOpType.subtract,
            )
            nc.vector.reduce_max(out=sc_t[:, 0:1], in_=pr[:, WD], axis=mybir.AxisListType.X)
            # DVE divide doesn't lower, so use IEEE reciprocal then multiply
            nc.vector.reciprocal(out=sc_t[:, 2:3], in_=sc_t[:, 0:1])
            nc.vector.tensor_scalar(
                out=sc_t[:, 3:4], in0=sc_t[:, 2:3], scalar1=fs, scalar2=-0.01,
                op0=mybir.AluOpType.mult, op1=mybir.AluOpType.add,
            )
            nc.vector.tensor_scalar(
                out=sc_t[:, 3:4], in0=sc_t[:, 3:4], scalar1=sc, scalar2=None,
                op0=mybir.AluOpType.min,
            )
            s_ap = sc_t[:, 3:4]
            nc.vector.tensor_scalar(
                out=pr[:, MN], in0=pr[:, CMIN], scalar1=s_ap, scalar2=None,
                op0=mybir.AluOpType.mult,
            )
            nc.vector.tensor_scalar(
                out=pr[:, MX], in0=pr[:, CMAX], scalar1=s_ap, scalar2=None,
                op0=mybir.AluOpType.mult,
            )
            rng_t = small.tile([P, 12], f32)
            nc.vector.tensor_tensor(
                out=rng_t[:, 0:3], in0=pr[:, MX], in1=pr[:, MN],
                op=mybir.AluOpType.subtract,
            )
            # t = fs - rng ; t0 = max(t - .001, 0) ; t1 = min(t + .001, 0)
            nc.vector.tensor_scalar(
                out=rng_t[:, 3:6], in0=rng_t[:, 0:3], scalar1=-1.0, scalar2=fs,
                op0=mybir.AluOpType.mult, op1=mybir.AluOpType.add,
            )
            nc.vector.tensor_scalar(
                out=rng_t[:, 6:9], in0=rng_t[:, 3:6], scalar1=-0.001, scalar2=0.0,
                op0=mybir.AluOpType.add, op1=mybir.AluOpType.max,
            )
            nc.vector.tensor_scalar(
                out=rng_t[:, 9:12], in0=rng_t[:, 3:6], scalar1=0.001, scalar2=0.0,
                op0=mybir.AluOpType.add, op1=mybir.AluOpType.min,
            )
            # off = (t0*j0 - mn) + t1*j1
            off_t = small.tile([P, 9], f32)
            nc.vector.tensor_tensor(
                out=off_t[:, 0:3], in0=rng_t[:, 6:9], in1=jit_t[:, 0:3],
                op=mybir.AluOpType.mult,
            )
            nc.vector.tensor_tensor(
                out=off_t[:, 3:6], in0=rng_t[:, 9:12], in1=jit_t[:, 3:6],
                op=mybir.AluOpType.mult,
            )
            nc.vector.tensor_tensor(
                out=off_t[:, 0:3], in0=off_t[:, 0:3], in1=pr[:, MN],
                op=mybir.AluOpType.subtract,
            )
            nc.vector.tensor_tensor(
                out=off_t[:, 0:3], in0=off_t[:, 0:3], in1=off_t[:, 3:6],
                op=mybir.AluOpType.add,
            )

            # --- transform: ccout = (ccraw - cmean) * s + off ---
            for c in range(3):
                nc.vector.tensor_scalar(
                    out=ccout[:, :, c], in0=ccraw[:, :, c],
                    scalar1=pr[:, c : c + 1], scalar2=s_ap,
                    op0=mybir.AluOpType.subtract, op1=mybir.AluOpType.mult,
                )
                nc.vector.tensor_scalar(
                    out=ccout[:, :, c], in0=ccout[:, :, c],
                    scalar1=off_t[:, c : c + 1], scalar2=None,
                    op0=mybir.AluOpType.add,
                )

            outc_3d = outc_d.ap().rearrange("(p a) c -> p a c", p=P)
            nc.sync.dma_start(out=outc_3d[:], in_=ccout[:])

    nc.compile()
    _CACHE[key] = nc
    return nc


def _plan(pid_full):
    """Per-core bank plan: sort each cluster's points by bank, pad cells.

    Returns (Jb tuple, per-core idx arrays [P, 8*Jtot] i16,
    per-core slot-of-original [P, PTS] i32, per-core dup (rows, counts))."""
    import hashlib

    key = hashlib.sha256(pid_full.tobytes()).hexdigest()
    if key in _PLAN_CACHE:
        return _PLAN_CACHE[key]

    MASK = (1 << BSHIFT) - 1
    per_core = []
    counts_all = []
    for k in range(NCORES):
        pk = pid_full[k * PPC : (k + 1) * PPC].reshape(P, PTS).astype(np.int64)
        bank = (pk >> BSHIFT).astype(np.int32)
        order = np.argsort(bank, axis=1, kind="stable")        # (P, PTS)
        sb = np.take_along_axis(bank, order, axis=1)
        sl = (np.take_along_axis(pk, order, axis=1) & MASK).astype(np.int32)
        cnt = np.zeros((P, BANKS), np.int32)
        for p in range(P):
            cnt[p] = np.bincount(sb[p], minlength=BANKS)
        counts_all.append(cnt)
        per_core.append((pk, order, sb, sl, cnt))

    cnt_max = np.max(np.stack([c for c in counts_all]), axis=(0, 1))  # (BANKS,)
    if np.any(np.min(np.stack(counts_all), axis=(0, 1)) == 0):
        _PLAN_CACHE[key] = None
        return None
    Jb = tuple(int(x) for x in cnt_max)
    Jtot = int(sum(Jb))
    offs = np.concatenate([[0], np.cumsum(cnt_max)]).astype(np.int64)  # (BANKS+1,)

    plans = []
    for k in range(NCORES):
        pk, order, sb, sl, cnt = per_core[k]
        idx_mat = np.zeros((P, Jtot), np.int16)
        slot_of = np.zeros((P, PTS), np.int32)
        dup_rows = []
        dup_cnts = []
        grp_start = np.zeros((P, BANKS), np.int64)
        for p in range(P):
            grp_start[p] = np.concatenate([[0], np.cumsum(cnt[p])[:-1]])
        # slot of sorted rank r: offs[bank] + (r - grp_start[bank])
        r = np.arange(PTS)[None, :]
        slot_sorted = offs[sb] + (r - np.take_along_axis(grp_start, sb, axis=1))
        np.put_along_axis(slot_of, order, slot_sorted.astype(np.int32), axis=1)
        # fill idx_mat: real entries then duplicate the cell's first entry
        pidx = np.arange(P)[:, None]
        idx_mat[pidx, slot_sorted] = sl.astype(np.int16)
        first = np.take_along_axis(sl, grp_start.astype(np.int64), axis=1)  # (P, BANKS)
        for b in range(BANKS):
            nb = cnt[:, b]
            pad = cnt_max[b] - nb                      # (P,)
            if pad.max() == 0:
                continue
            # columns offs[b]+nb ... offs[b]+Jb[b]-1 get first[:, b]
            width = int(cnt_max[b])
            cols = offs[b] + np.arange(width)[None, :]            # (1, width)
            padmask = np.arange(width)[None, :] >= nb[:, None]    # (P, width)
            vals = np.where(padmask, first[:, b : b + 1], idx_mat[pidx, cols])
            idx_mat[pidx, cols] = vals.astype(np.int16)
            rows_global = (b << BSHIFT) + first[:, b].astype(np.int64)  # (P,)
            dup_rows.append(rows_global)
            dup_cnts.append(pad)
        # wrapped int16 layout per SPAN-chunk call
        wrapped = np.zeros((P, 8 * Jtot), np.int16)
        goff = 0
        for b in range(BANKS):
            done = 0
            while done < Jb[b]:
                span = min(SPAN, Jb[b] - done)
                iv = idx_mat[:, goff : goff + span].T.ravel()     # (span*P,)
                w = iv.reshape(span * 8, 16).T                    # (16, span*8)
                wrapped[:, 8 * goff : 8 * (goff + span)] = np.tile(w, (8, 1))
                goff += span
                done += span
        plans.append(
            {
                "idx": np.ascontiguousarray(wrapped),
                "slot_of": slot_of,
                "dup_rows": np.stack(dup_rows, 1) if dup_rows else np.zeros((P, 0), np.int64),
                "dup_cnts": np.stack(dup_cnts, 1) if dup_cnts else np.zeros((P, 0), np.int64),
            }
        )
    out = (Jb, plans)
    _PLAN_CACHE[key] = out
    return out


def _reference_numpy(clusters_idx, clusters_offset, feats, coords, jitter, fullscale, scale):
    seg = clusters_idx[:, 0].astype(np.int64)
    pid = clusters_idx[:, 1].astype(np.int64)
    nC = clusters_offset.shape[0] - 1
    fs = np.float32(fullscale)
    cf = feats[pid]
    cc = coords[pid].astype(np.float32)
    cnt = np.diff(clusters_offset).astype(np.float32)[:, None]
    sums = np.zeros((nC, 3), np.float32)
    np.add.at(sums, seg, cc)
    cmean = sums / np.maximum(cnt, 1.0)
    ccc = cc - cmean[seg]
    cmin = np.full((nC, 3), np.inf, np.float32)
    cmax = np.full((nC, 3), -np.inf, np.float32)
    np.minimum.at(cmin, seg, ccc)
    np.maximum.at(cmax, seg, ccc)
    cscale = 1.0 / ((cmax - cmin) / fs).max(axis=1) - np.float32(0.01)
    cscale = np.minimum(cscale, np.float32(scale)).astype(np.float32)
    mn = cmin * cscale[:, None]
    mx = cmax * cscale[:, None]
    ccc = ccc * cscale[seg][:, None]
    rng = mx - mn
    off = (-mn + np.maximum(fs - rng - 0.001, 0.0) * jitter[0]
           + np.minimum(fs - rng + 0.001, 0.0) * jitter[1]).astype(np.float32)
    ccc = ccc + off[seg]
    return np.concatenate([cf, ccc], axis=1).astype(np.float32)


def kernel(clusters_idx, clusters_offset, feats, coords, jitter, fullscale, scale):
    clusters_idx = np.asarray(clusters_idx)
    clusters_offset = np.asarray(clusters_offset)
    feats = np.asarray(feats, dtype=np.float32)
    coords = np.asarray(coords, dtype=np.float32)
    jitter = np.asarray(jitter, dtype=np.float32)

    fs = float(np.asarray(fullscale).item()) if not isinstance(fullscale, (int, float)) else float(fullscale)
    sc = float(np.asarray(scale).item()) if not isinstance(scale, (int, float)) else float(scale)

    uniform = (
        clusters_idx.shape == (S, 2)
        and clusters_offset.shape == (NCLUSTER + 1,)
        and feats.shape == (N, C)
        and coords.shape == (N, 3)
        and np.array_equal(
            clusters_offset,
            np.arange(NCLUSTER + 1, dtype=np.int64) * PTS,
        )
        and np.array_equal(
            clusters_idx[:, 0],
            np.repeat(np.arange(NCLUSTER, dtype=np.int64), PTS),
        )
    )
    if not uniform:
        return _reference_numpy(
            clusters_idx, clusters_offset, feats, coords, jitter, fs, sc
        )

    pid_full = np.ascontiguousarray(clusters_idx[:, 1].astype(np.int32))
    plan = _plan(pid_full)
    if plan is None:
        return _reference_numpy(
            clusters_idx, clusters_offset, feats, coords, jitter, fs, sc
        )
    Jb, plans = plan
    Jtot = int(sum(Jb))

    nc = _build_program(fs, sc, Jb)

    table = np.zeros((N, EROW), dtype=np.float16)
    table[:, :C] = feats
    table[:, C : C + 3] = coords
    coords16 = table[:, C : C + 3].astype(np.float32)

    in_maps = []
    for k in range(NCORES):
        pl = plans[k]
        corr = (
            coords16[pl["dup_rows"]] * pl["dup_cnts"][:, :, None]
        ).sum(axis=1).astype(np.float32)
        in_maps.append(
            {"table": table, "idxs": pl["idx"], "jit": jitter, "corr": corr}
        )

    res = bass_utils.run_bass_kernel_spmd(
        nc, in_maps, core_ids=list(range(NCORES)), trace=TRACE
    )
    global LAST_RESULTS
    LAST_RESULTS = res

    out = np.empty((S, ROW), dtype=np.float32)
    pidx = np.arange(P)[:, None]
    for k in range(NCORES):
        sl = slice(k * PPC, (k + 1) * PPC)
        slot_of = plans[k]["slot_of"]
        oh = res.results[k]["outh"].reshape(P, Jtot, ROW)
        oc = res.results[k]["outc"].reshape(P, Jtot, 3)
        out[sl, :C] = oh[pidx, slot_of, :C].reshape(PPC, C).astype(np.float32)
        out[sl, C:] = oc[pidx, slot_of].reshape(PPC, 3)
    return out
